# revision 8
# baseline (speedup 1.0000x reference)
"""Trainium2 Bass kernel for nn_MultiHeadAttention_10960756539999.

MHA: inp [2, 2048, 768], 12 heads, head_dim 64, Wqkv [768, 2304] (per-head
192-col slabs laid out [Q|K|V]), Wproj [768, 768].

Sharding: 24 (batch, head) pairs -> 3 heads per core; cores 0-3 take batch 0,
cores 4-7 take batch 1. Each core computes QKV^T for its heads from x^T,
attention fully on-chip (softmax over the free axis of scores^T, no max
subtraction -- scores are ~N(0,1)), and a row-sharded partial projection
out_heads @ Wproj[rows]. The host sums the 4 per-batch partials and adds
bproj.

All tensors are bf16 in DRAM/SBUF (f32 PSUM accumulation). The cost model
charges a matmul its OUTPUT free size per contraction tile, so attV runs
"transposed": exp tiles are the stationary operand and V the moving one,
accumulating [128 queries, 64 dims + 1 denom] per query-tile (free 65/tile
instead of 1024).  That halves attV's PE rows vs the [dims, queries]
layout and lands the softmax denominator in a PSUM *column*, so
normalization is a DVE reciprocal + per-partition tensor_scalar multiply
(no PE broadcast matmuls).  The normalized [query, dim] tiles for heads
0/1 of a query half pair up into [128,128] blocks that a PE
identity-transpose flips into the [dims, tokens] oT layout the projection
needs (h2 rides with a junk half).  Transposes and all non-attention
matmuls (V staging, heads 1-2 QK^T, the projection) are "filler units"
emitted inside the exp-bound attention j-loops so the PE chews them while
waiting on exp.  Pass order is head-major with h2 last; proj of the first
query half runs as filler in the last pass, proj of the second half is
the tail.
"""

import os
import sys

import numpy as np
import ml_dtypes

try:
    import concourse.bass as bass
except ImportError:  # harness runs from a bare directory
    sys.path.insert(0, "/opt/trn_rl_repo")
    import concourse.bass as bass

import concourse.tile as tile
from concourse import bacc, mybir
from concourse.bass_utils import run_bass_kernel_spmd
from concourse.masks import make_identity

F32 = mybir.dt.float32
BF16 = mybir.dt.bfloat16
AF = mybir.ActivationFunctionType
NPBF16 = ml_dtypes.bfloat16

NH = 12          # total heads
D = 64           # head dim
S = 2048         # sequence length
NI = 768         # model dim
NB = 2           # batch
NCORES = 8
HPC = 3          # heads per core
CPB = NCORES // NB   # cores per batch
KC = NI // 128   # contraction chunks for the 768 dim
NT = S // 128    # 128-row tiles along tokens/keys
HB = S // 2      # 1024: half the token/query axis
NQT = HB // 128  # 8 query tiles per half
SCALE = float(1.0 / np.sqrt(NI / NH))  # 1/8

# filled by kernel() for test.py to report
last_results = None

_cache = {}


def _build_nc(has_bias: bool):
    nc = bacc.Bacc("TRN2", target_bir_lowering=False, debug=False,
                   num_devices=NCORES)

    xT_d = nc.dram_tensor("xT", [NI, S], BF16, kind="ExternalInput")
    wqk_d = nc.dram_tensor("wqk", [NI, HPC * 128], BF16, kind="ExternalInput")
    # wv padded to 256 cols so each DMA descriptor is 512B (full-rate)
    wv_d = nc.dram_tensor("wv", [NI, 256], BF16, kind="ExternalInput")
    wp_d = nc.dram_tensor("wp", [HPC * D, NI], BF16, kind="ExternalInput")
    if has_bias:
        # cols 2h = bq_h, 2h+1 = bk_h (64 rows each); bv packed per-head
        bqk_d = nc.dram_tensor("bqk", [D, 2 * HPC], F32, kind="ExternalInput")
        bv_d = nc.dram_tensor("bv", [HPC * D], F32, kind="ExternalInput")
    out_d = nc.dram_tensor("out", [S, NI], BF16, kind="ExternalOutput")

    with tile.TileContext(nc) as tc:
        with (
            tc.tile_pool(name="const", bufs=1) as constp,
            tc.tile_pool(name="expp", bufs=6) as expp,
            tc.tile_pool(name="opool", bufs=1) as opool,
            tc.tile_pool(name="rwork", bufs=2) as rwork,
            # PSUM (8 banks x 2KB): tag A = 2 slots x 2 banks (QK^T h0 in
            # the load phase, then the 2-deep scores rotation, then proj
            # tail), tag B = 1 slot x 2 banks (V group 0, then the per-pass
            # [q, dims] attV accumulator), tag C/D = 1 bank each (in-pass
            # filler units and oT transposes).
            tc.tile_pool(name="ps", bufs=2, space="PSUM") as psp,
            tc.tile_pool(name="dramp", bufs=2, space="DRAM") as dramp,
        ):
            # ---- SBUF tensors ----
            xT = constp.tile([128, KC, S], BF16, tag="xT")
            wqk = constp.tile([128, KC, HPC * 128], BF16, tag="wqk")
            wv = constp.tile([128, KC, 256], BF16, tag="wv")
            wp01 = constp.tile([128, NI], BF16, tag="wp01")
            wp2 = constp.tile([D, NI], BF16, tag="wp2")
            qq = [constp.tile([D, S], BF16, tag=f"qq{h}", name=f"qq{h}")
                  for h in range(HPC)]
            kk = [constp.tile([D, S], BF16, tag=f"kk{h}", name=f"kk{h}")
                  for h in range(HPC)]
            # vaug layout: [128, h, NT*65]; per key-chunk j the slab
            # [:, h, 65j : 65j+65] is [V_h(chunk j) | ones].
            vaug = constp.tile([128, HPC, NT * 65], BF16, tag="vaug")
            oT01 = opool.tile([128, S], BF16, tag="oT01")
            oT2 = opool.tile([D, S], BF16, tag="oT2")
            # normalized [query, dim] staging per query half: heads 0/1
            # pack into cols 0:64 / 64:128 per 128-query tile; head 2 uses
            # its own slab with a junk upper half (never read after the
            # transpose).
            sb01 = [constp.tile([128, NQT, 128], BF16, tag=f"sb01_{qh}",
                                name=f"sb01_{qh}") for qh in range(2)]
            sb2 = [constp.tile([128, NQT, 128], BF16, tag=f"sb2_{qh}",
                               name=f"sb2_{qh}") for qh in range(2)]

            # ---- input DMAs. The first exp only needs Q/K of head 0 for
            # query half 0, i.e. xT columns 0:1024 over all 6 contraction
            # chunks plus the h0 wqk slabs. SP queue: xT half 0 with wv
            # interleaved (the V staging follows right behind the QK load),
            # then xT half 1, then the remaining weights. ACT queue (idle
            # until the first exp): the small h0 QK weight slabs.
            xT_src = xT_d[:].rearrange("(c p) s -> p c s", p=128)
            wqk_src = wqk_d[:].rearrange("(c p) m -> p c m", p=128)
            wv_src = wv_d[:].rearrange("(c p) m -> p c m", p=128)
            if has_bias:
                bqk = constp.tile([D, 2 * HPC], F32, tag="bqk")
                nc.scalar.dma_start(out=bqk, in_=bqk_d[:])
                bvb = constp.tile([128, HPC * D], F32, tag="bvb")
                bv_ap = bv_d[:]
                bv_bcast = bass.AP(
                    tensor=bv_ap.tensor, offset=bv_ap.offset,
                    ap=[[0, 128]] + [list(p) for p in bv_ap.ap])
                nc.scalar.dma_start(out=bvb, in_=bv_bcast)
                bvb3 = bvb.rearrange("p (h d) -> p h d", d=D)
            for c in range(KC):
                nc.scalar.dma_start(out=wqk[:, c, 0:128],
                                    in_=wqk_src[:, c, 0:128])
            for c in range(KC):
                if c < KC - 1:
                    nc.sync.dma_start(out=xT[:, c, 0:HB],
                                      in_=xT_src[:, c, 0:HB])
                else:
                    # split the gating chunk so the first QK^T matmul can
                    # fire before the full half lands
                    nc.sync.dma_start(out=xT[:, c, 0:512],
                                      in_=xT_src[:, c, 0:512])
                    nc.sync.dma_start(out=xT[:, c, 512:HB],
                                      in_=xT_src[:, c, 512:HB])
                nc.sync.dma_start(out=wv[:, c, :], in_=wv_src[:, c, :])
            for c in range(KC):
                nc.sync.dma_start(out=xT[:, c, HB:S], in_=xT_src[:, c, HB:S])
            for c in range(KC):
                nc.sync.dma_start(out=wqk[:, c, 128:HPC * 128],
                                  in_=wqk_src[:, c, 128:HPC * 128])
            nc.sync.dma_start(out=wp01, in_=wp_d[0:128, :])
            nc.sync.dma_start(out=wp2, in_=wp_d[128:HPC * D, :])

            # ---- ACT warmup: trigger the Exp table load at t=0 ----
            warm_in = constp.tile([1, 2], F32, tag="warm_in")
            warm_out = constp.tile([1, 2], F32, tag="warm_out")
            nc.vector.memset(warm_in, 0.0)
            nc.scalar.activation(warm_out, warm_in, AF.Exp, scale=1.0)

            # PE-warmup scratch first in the DVE queue so warm matmuls
            # start immediately
            scr = constp.tile([128, 512], BF16, tag="scr")
            nc.vector.memset(scr, 0.0)

            # identity for the PE transposes (gpsimd builds it off the
            # critical path)
            ident = constp.tile([128, 128], BF16, tag="ident")
            make_identity(nc, ident)

            # junk halves of the h2 staging slabs must still be defined
            # bits for the transpose reads
            for qh in range(2):
                nc.gpsimd.memset(sb2[qh][:, :, D:128], 0.0)

            # ones columns of vaug
            ones_sb = constp.tile([128, NT, 1], BF16, tag="ones")
            nc.vector.memset(ones_sb, 1.0)
            v4 = vaug.rearrange("p h (t c) -> p h t c", c=65)
            for h in range(HPC):
                nc.vector.tensor_copy(v4[:, h, :, 64:65], ones_sb)

            # ---- copy helpers (PSUM sources: DVE or ACT only --
            # GPSIMD cannot access PSUM) ----
            def qk_copy(h, sl, src, k_eng="v"):
                # src [128, len] psum (q rows 0:64, k rows 64:128); the k
                # copy shifts partitions 64-127 down to 0-63
                if has_bias:
                    nc.vector.tensor_scalar_add(
                        qq[h][:, sl], src[0:D, :], bqk[:, 2 * h:2 * h + 1])
                    nc.vector.tensor_scalar_add(
                        kk[h][:, sl], src[D:128, :],
                        bqk[:, 2 * h + 1:2 * h + 2])
                    return
                nc.vector.tensor_copy(qq[h][:, sl], src[0:D, :])
                if k_eng == "a":
                    nc.scalar.copy(kk[h][:, sl], src[D:128, :])
                else:
                    nc.vector.tensor_copy(kk[h][:, sl], src[D:128, :])

            def v_copy(pv, ti, t):
                # pv [128, n, 256] psum; one fused copy into all 3 heads'
                # vaug slabs
                dst = v4[:, :, t, 0:D]
                src = pv[:, ti, 0:HPC * D].rearrange("p (h d) -> p h d", d=D)
                if has_bias:
                    nc.vector.tensor_add(dst, src, bvb3)
                else:
                    nc.vector.tensor_copy(dst, src)

            # ---- PE p-state warmup into the bank the first real
            # start=True matmul will clear ----
            qktt = psp.tile([128, HB], F32, tag="A", name="qkt0")
            for _ in range(6):
                nc.tensor.matmul(qktt[:, 0:512], scr[:, 0:128], scr,
                                 start=True, stop=True,
                                 skip_group_check=True)

            # ---- load phase: QK^T(h0, query half 0) tracks the xT chunk
            # stream; half 1 runs as qkt_piece fillers inside pass 0 once
            # the xT half-1 chunks land. The V matmuls (wv rides between
            # the xT half-0 chunks) are emitted between the scores
            # prologue and the j-loop ----
            pv0 = psp.tile([128, 4, 256], F32, tag="B", name="pv0", bufs=1)
            for c in range(KC):
                for n in range(2):
                    nc.tensor.matmul(
                        qktt[:, n * 512:(n + 1) * 512],
                        wqk[:, c, 0:128], xT[:, c, n * 512:(n + 1) * 512],
                        start=(c == 0), stop=(c == KC - 1),
                        skip_group_check=True)
            # gate sc(0)/sc(1): q copies on DVE; k cols 0:512 (all the
            # early scores need) on the idle ACT, k cols 512:1024 on DVE
            # behind q so exp(0) isn't queued behind it
            nc.vector.tensor_copy(qq[0][:, 0:512], qktt[0:D, 0:512])
            nc.vector.tensor_copy(qq[0][:, 512:HB], qktt[0:D, 512:HB])
            nc.scalar.copy(kk[0][:, 0:512], qktt[D:128, 0:512])
            nc.vector.tensor_copy(kk[0][:, 512:HB], qktt[D:128, 512:HB])

            def pv_mms(pv, base):
                for c in range(KC):
                    for ti in range(pv.shape[1]):
                        t = base + ti
                        nc.tensor.matmul(
                            pv[:, ti, 0:HPC * D],
                            xT[:, c, t * 128:(t + 1) * 128],
                            wv[:, c, 0:HPC * D],
                            start=(c == 0 and ti % 2 == 0),
                            stop=(c == KC - 1), skip_group_check=True)

            def p0_preloop():
                # V tiles 0-3: matmuls on the PE while the h0 copies and
                # the first exps run; drains in consumption order. Tiles
                # 4-7 ride as slot-0/1 fillers, 8-15 as later fillers.
                pv_mms(pv0, 0)
                for ti in range(4):
                    v_copy(pv0, ti, ti)
                if has_bias:
                    nc.vector.tensor_scalar_add(
                        qq[0][:, 0:HB], qq[0][:, 0:HB], bqk[:, 0:1])
                    nc.vector.tensor_scalar_add(
                        kk[0][:, 0:HB], kk[0][:, 0:HB], bqk[:, 1:2])

            def pv_unit(base, tag):
                def emit():
                    pv = psp.tile([128, 2, 256], F32, tag=tag,
                                  name=f"pv{base}", bufs=1)
                    pv_mms(pv, base)
                    for ti in range(pv.shape[1]):
                        v_copy(pv, ti, base + ti)
                return emit

            # ---- filler units: each fits a single PSUM bank, and
            # alternates between the C and D slots so one unit's drain
            # copy overlaps the next unit's matmuls ----
            def v_unit(t, tag):
                def emit():
                    pvu = psp.tile([128, 1, 256], F32, tag=tag, bufs=1,
                                   name=f"vu{t}")
                    for c in range(KC):
                        nc.tensor.matmul(
                            pvu[:, 0, 0:HPC * D],
                            xT[:, c, t * 128:(t + 1) * 128],
                            wv[:, c, 0:HPC * D],
                            start=(c == 0), stop=(c == KC - 1))
                    v_copy(pvu, 0, t)
                return emit

            def qkt_piece(h, half, n, tag):
                # three emissions (consecutive filler slots) so the 6-chunk
                # matmul burst doesn't starve the exp stream
                state = {}
                sl = slice(half * HB + n * 512, half * HB + (n + 1) * 512)

                def mm_range(c0, c1):
                    for c in range(c0, c1):
                        nc.tensor.matmul(
                            state["pc"], wqk[:, c, h * 128:(h + 1) * 128],
                            xT[:, c, sl],
                            start=(c == 0), stop=(c == KC - 1))

                def emit_a():
                    state["pc"] = psp.tile([128, 512], F32, tag=tag, bufs=1,
                                           name=f"qp{h}_{half}_{n}")
                    mm_range(0, 2)

                def emit_b():
                    mm_range(2, 4)

                def emit_c():
                    mm_range(4, KC)
                    qk_copy(h, sl, state["pc"])
                return emit_a, emit_b, emit_c

            # ---- oT transpose units: one [128,128] normalized [q, d]
            # block -> PE identity transpose -> [d, q] drain into oT ----
            def t_unit(h01, qh, qts, tag, eng="v"):
                # h01: True = heads 0/1 pair into oT01, False = head 2
                src = sb01[qh] if h01 else sb2[qh]

                def emit():
                    for i, qt in enumerate(qts):
                        tp = psp.tile([128, 128], BF16,
                                      tag=(tag if i % 2 == 0 else
                                           ("D" if tag == "C" else "C")),
                                      bufs=1, name=f"tp{int(h01)}{qh}{qt}")
                        nc.tensor.transpose(tp, src[:, qt, :], ident)
                        dst_sl = slice(qh * HB + qt * 128,
                                       qh * HB + (qt + 1) * 128)
                        if h01:
                            dst = oT01[:, dst_sl]
                            srcp = tp
                        else:
                            dst = oT2[:, dst_sl]
                            srcp = tp[0:D, :]
                        if eng == "a":
                            nc.scalar.copy(dst, srcp)
                        else:
                            nc.vector.tensor_copy(dst, srcp)
                return emit

            # ---- projection ----
            out_dst = out_d[:].rearrange("(t p) o -> t p o", p=128)
            ostage = {}

            def ost_for(t):
                if t not in ostage:
                    ostage[t] = rwork.tile([128, NI], BF16, tag="ostage",
                                           bufs=6, name=f"ost{t}")
                return ostage[t]

            def proj_piece(t, r, tag, eng):
                # one output region (r=0: cols 0:512, r=1: 512:768) of proj
                # tile t through a single-bank slot
                def emit():
                    n0, n1 = ((0, 512), (512, NI))[r]
                    pp = psp.tile([128, 512], F32, tag=tag, bufs=1,
                                  name=f"pp{t}_{r}")
                    o01 = oT01[:, t * 128:(t + 1) * 128]
                    o2 = oT2[:, t * 128:(t + 1) * 128]
                    nc.tensor.matmul(pp[:, 0:n1 - n0], o01, wp01[:, n0:n1],
                                     start=True, stop=False)
                    nc.tensor.matmul(pp[:, 0:n1 - n0], o2, wp2[:, n0:n1],
                                     start=False, stop=True)
                    ost = ost_for(t)
                    if eng == "a":
                        nc.scalar.copy(ost[:, n0:n1], pp[:, 0:n1 - n0])
                    else:
                        nc.vector.tensor_copy(ost[:, n0:n1], pp[:, 0:n1 - n0])
                    nc.sync.dma_start(out=out_dst[t][:, n0:n1],
                                      in_=ost[:, n0:n1])
                return emit

            def proj_mm1(t, tag):
                pp = psp.tile([128, HB], F32, tag=tag,
                              bufs=(None if tag == "A" else 1),
                              name=f"pp{t}")
                o01 = oT01[:, t * 128:(t + 1) * 128]
                for n0, n1 in ((0, 512), (512, NI)):
                    nc.tensor.matmul(pp[:, n0:n1], o01, wp01[:, n0:n1],
                                     start=True, stop=False)
                return pp

            def proj_mm2(t, pp, eng):
                o2 = oT2[:, t * 128:(t + 1) * 128]
                for n0, n1 in ((0, 512), (512, NI)):
                    nc.tensor.matmul(pp[:, n0:n1], o2, wp2[:, n0:n1],
                                     start=False, stop=True)
                ost = ost_for(t)
                if eng == "a":
                    nc.scalar.copy(ost, pp[:, 0:NI])
                else:
                    nc.vector.tensor_copy(ost, pp[:, 0:NI])
                nc.sync.dma_start(out=out_dst[t], in_=ost)

            def proj_unit(t, tag, eng):
                def emit():
                    proj_mm2(t, proj_mm1(t, tag), eng)
                return emit

            # ---- attention pass ----
            def sc_tile(h, qh, j):
                sc = psp.tile([128, HB], F32, tag="A", name="sc")
                klhs = kk[h][:, j * 128:(j + 1) * 128]
                for n in range(2):
                    sl = slice(qh * HB + n * 512, qh * HB + (n + 1) * 512)
                    nc.tensor.matmul(
                        sc[:, n * 512:(n + 1) * 512], klhs, qq[h][:, sl])
                return sc

            prologues = {}

            def attention_pass(h, qh, filler, last=False, pre_loop=None,
                               nxt=None):
                # acc2 [128 queries, 8 qt, 128]: per query tile, cols 0:64
                # hold out_h[q, d] unnormalized and col 64 the softmax
                # denominator (ones column of vaug); cols 65:128 are pad
                # so each qt slab is 512B and 4 slabs fill a PSUM bank.
                acc2 = psp.tile([128, NQT, 128], F32, tag="B",
                                name=f"acc{h}{qh}", bufs=1)

                sc_q = prologues.pop((h, qh), None)
                if sc_q is None:
                    sc_q = [sc_tile(h, qh, 0)]
                sc_q.append(sc_tile(h, qh, 1))
                if pre_loop is not None:
                    pre_loop()
                fq = list(filler)
                for j in range(NT):
                    # prefire the next pass's first score tile ahead of the
                    # final exp/attV so the next exp(0) has zero gap: its A
                    # slot is free as soon as exp(j=14) has read it
                    if j == NT - 1 and nxt is not None:
                        prologues[nxt] = [sc_tile(*nxt, 0)]
                    ex = expp.tile([128, HB], BF16, tag="exp")
                    nc.scalar.activation(ex, sc_q.pop(0), AF.Exp,
                                         scale=SCALE)
                    if j + 2 < NT:
                        sc_q.append(sc_tile(h, qh, j + 2))
                    # filler between the scores matmul and attV so the PE
                    # has work while exp(j) is still running
                    for _ in range(2):
                        if fq and fq[0][0] <= j:
                            fq.pop(0)[1]()
                        else:
                            break
                    vslab = vaug[:, h, j * 65:j * 65 + 65]
                    for qt in range(NQT):
                        nc.tensor.matmul(
                            acc2[:, qt, 0:65],
                            ex[:, qt * 128:(qt + 1) * 128], vslab,
                            start=(j == 0 and qt % 4 == 0),
                            stop=(j == NT - 1), skip_group_check=True)
                for _, f in fq:  # leftovers (shouldn't happen)
                    f()
                return acc2

            def norm2(h, qh, acc2):
                # acc2 col 64 holds the per-query denominator: DVE
                # reciprocal, then scale cols 0:64 into the bf16 [q, d]
                # staging slab (single-op psum release per qt slice).
                rcp = rwork.tile([128, NQT, 1], F32, tag="rcp", bufs=2)
                nc.vector.reciprocal(rcp, acc2[:, :, 64:65])
                dst = sb01[qh] if h < 2 else sb2[qh]
                col = h * D if h < 2 else 0
                for qt in range(NQT):
                    nc.vector.tensor_scalar_mul(
                        dst[:, qt, col:col + D], acc2[:, qt, 0:D],
                        rcp[:, qt])

            # ---- pass schedule (head-major, h2 last) ----
            # p0 (h0,q0): k(h0) half1 copy + V tiles 8-15
            # p1 (h0,q1): QK^T(h1) pieces
            # p2 (h1,q0): QK^T(h2) pieces 0-1
            # p3 (h1,q1): QK^T(h2) pieces 2-3 + oT01 transpose (q-half 0)
            # p4 (h2,q0): oT01 transpose (q-half 1)
            # p5 (h2,q1): oT2 transpose (q-half 0) + proj of query half 0
            # tail: oT2 transpose (q-half 1) + proj of query half 1
            cd = ("C", "D")

            def pieces(h, entries, j0, dj):
                out = []
                for i, (hf, n) in enumerate(entries):
                    for k, e in enumerate(qkt_piece(h, hf, n, cd[i % 2])):
                        out.append((j0 + dj * i + k, e))
                return out

            # p0: V tiles 4-7 at slots 0/1, then the h0 half-1 QK pieces
            # (keys 1024:1536 must land before sc(8) is emitted at j=6,
            # keys 1536:2048 before sc(12) at j=10 -- emitting them later
            # would deadlock the in-order PE queue), V tiles 8-15 spread
            # so tile t lands before attV consumes it at j=t.
            p0 = ([(0, pv_unit(4, "C")), (1, pv_unit(6, "D"))]
                  + pieces(0, ((1, 0),), 3, 1)
                  + [(6 + i, v_unit(8 + i, cd[(i + 1) % 2]))
                     for i in range(8)]
                  + pieces(0, ((1, 1),), 7, 1))
            p0.sort(key=lambda x: x[0])
            p1 = pieces(1, ((0, 0), (0, 1), (1, 0), (1, 1)), 3, 3)
            p2 = pieces(2, ((0, 0), (0, 1)), 3, 5)
            p3 = pieces(2, ((1, 0), (1, 1)), 3, 5)
            p3 += [(13 + i, t_unit(True, 0, qts, cd[i % 2]))
                   for i, qts in enumerate(((0, 1, 2), (3, 4, 5), (6, 7)))]
            p4 = [(1 + 2 * i, t_unit(True, 1, qts, cd[i % 2]))
                  for i, qts in enumerate(((0, 1, 2), (3, 4, 5), (6, 7)))]
            p5 = [(i, t_unit(False, 0, qts, cd[i % 2]))
                  for i, qts in enumerate(((0, 1, 2), (3, 4, 5), (6, 7)))]
            # last proj tile of the q0 half is emitted in the tail so its
            # drain copies don't queue ahead of the tail reciprocal
            p5 += [(3 + i, proj_piece(i // 2, i % 2, cd[i % 2], "v"))
                   for i in range(14)]
            acc00 = attention_pass(0, 0, p0, pre_loop=p0_preloop,
                                   nxt=(0, 1))
            acc01 = attention_pass(0, 1, p1,
                                   pre_loop=lambda: norm2(0, 0, acc00),
                                   nxt=(1, 0))
            acc10 = attention_pass(1, 0, p2,
                                   pre_loop=lambda: norm2(0, 1, acc01),
                                   nxt=(1, 1))
            acc11 = attention_pass(1, 1, p3,
                                   pre_loop=lambda: norm2(1, 0, acc10),
                                   nxt=(2, 0))
            acc20 = attention_pass(2, 0, p4,
                                   pre_loop=lambda: norm2(1, 1, acc11),
                                   nxt=(2, 1))
            acc_last = attention_pass(2, 1, p5,
                                      pre_loop=lambda: norm2(2, 0, acc20),
                                      last=True)

            # ---- tail: normalize h2/q1, transpose it into oT2, and
            # project tiles 8-15. Each proj tile 8+qt only needs oT2
            # query tile qt, so the tail runs a per-qt pipeline:
            # scale (DVE) -> PE transpose -> drain (DVE) -> proj matmuls
            # (PE) -> copy split ACT/DVE -> output DMA. ACT is idle after
            # the last exp, so it takes the larger copy half.
            pp_pre = {t: proj_mm1(t, "A") for t in (8, 9)}
            rcp_t = rwork.tile([128, NQT, 1], F32, tag="rcp", bufs=2)
            nc.vector.reciprocal(rcp_t, acc_last[:, :, 64:65])
            proj_piece(7, 0, "C", "a")()
            proj_piece(7, 1, "D", "a")()

            def proj_drain_split(t, pp):
                ost = ost_for(t)
                nc.scalar.copy(ost[:, 0:512], pp[:, 0:512])
                nc.vector.tensor_copy(ost[:, 512:NI], pp[:, 512:NI])
                nc.sync.dma_start(out=out_dst[t], in_=ost)

            for qt in range(NQT):
                nc.vector.tensor_scalar_mul(
                    sb2[1][:, qt, 0:D], acc_last[:, qt, 0:D], rcp_t[:, qt])
                tp = psp.tile([128, 128], BF16, tag=cd[qt % 2], bufs=1,
                              name=f"tpt{qt}")
                nc.tensor.transpose(tp, sb2[1][:, qt, :], ident)
                nc.vector.tensor_copy(
                    oT2[:, HB + qt * 128:HB + (qt + 1) * 128], tp[0:D, :])
                t8 = 8 + qt
                if t8 in pp_pre:
                    pp = pp_pre[t8]
                    o2 = oT2[:, t8 * 128:(t8 + 1) * 128]
                    for n0, n1 in ((0, 512), (512, NI)):
                        nc.tensor.matmul(pp[:, n0:n1], o2, wp2[:, n0:n1],
                                         start=False, stop=True)
                else:
                    pp = proj_mm1(t8, ("B", "A", "A", "B", "A", "A")[qt - 2])
                    o2 = oT2[:, t8 * 128:(t8 + 1) * 128]
                    for n0, n1 in ((0, 512), (512, NI)):
                        nc.tensor.matmul(pp[:, n0:n1], o2, wp2[:, n0:n1],
                                         start=False, stop=True)
                proj_drain_split(t8, pp)

    nc.compile()
    return nc


def _get_nc(has_bias: bool):
    if has_bias not in _cache:
        _cache[has_bias] = _build_nc(has_bias)
    return _cache[has_bias]


def kernel(inp, Wqkv, bqkv, Wproj, bproj):
    global last_results
    inp = np.ascontiguousarray(np.asarray(inp, dtype=np.float32))
    Wqkv = np.asarray(Wqkv, dtype=np.float32)
    bqkv = np.asarray(bqkv, dtype=np.float32)
    Wproj = np.asarray(Wproj, dtype=np.float32)
    bproj = np.asarray(bproj, dtype=np.float32)
    assert inp.shape == (NB, S, NI), inp.shape

    has_bias = bool(np.any(bqkv))
    nc = _get_nc(has_bias)

    xTs = [np.ascontiguousarray(inp[b].T).astype(NPBF16) for b in range(NB)]

    in_maps = []
    for core in range(NCORES):
        b = core // CPB
        heads = [(core % CPB) * HPC + i for i in range(HPC)]
        wqk = np.empty((NI, HPC * 128), np.float32)
        wvm = np.zeros((NI, 256), np.float32)
        wp = np.empty((HPC * D, NI), np.float32)
        for i, h in enumerate(heads):
            base = h * 3 * D
            wqk[:, i * 128:i * 128 + D] = Wqkv[:, base:base + D]
            wqk[:, i * 128 + D:(i + 1) * 128] = Wqkv[:, base + D:base + 2 * D]
            wvm[:, i * D:(i + 1) * D] = Wqkv[:, base + 2 * D:base + 3 * D]
            wp[i * D:(i + 1) * D, :] = Wproj[h * D:(h + 1) * D, :]
        m = {"xT": xTs[b], "wqk": wqk.astype(NPBF16),
             "wv": wvm.astype(NPBF16), "wp": wp.astype(NPBF16)}
        if has_bias:
            bqk = np.empty((D, 2 * HPC), np.float32)
            bv = np.empty((HPC * D,), np.float32)
            for i, h in enumerate(heads):
                base = h * 3 * D
                bqk[:, 2 * i] = bqkv[base:base + D]
                bqk[:, 2 * i + 1] = bqkv[base + D:base + 2 * D]
                bv[i * D:(i + 1) * D] = bqkv[base + 2 * D:base + 3 * D]
            m["bqk"] = bqk
            m["bv"] = bv
        in_maps.append(m)

    res = run_bass_kernel_spmd(nc, in_maps, core_ids=list(range(NCORES)))
    last_results = res

    out = np.zeros((NB, S, NI), np.float32)
    for core in range(NCORES):
        out[core // CPB] += np.asarray(res.results[core]["out"],
                                       dtype=np.float32)
    out += bproj
    return out


# revision 13
# speedup vs baseline: 1.0183x; 1.0183x over previous
"""Trainium2 Bass kernel for nn_MultiHeadAttention_10960756539999.

MHA: inp [2, 2048, 768], 12 heads, head_dim 64, Wqkv [768, 2304] (per-head
192-col slabs laid out [Q|K|V]), Wproj [768, 768].

Sharding: 24 (batch, head) pairs -> 3 heads per core; cores 0-3 take batch 0,
cores 4-7 take batch 1. Each core computes QKV^T for its heads from x^T,
attention fully on-chip (softmax over the free axis of scores^T, no max
subtraction -- scores are ~N(0,1)), and a row-sharded partial projection
out_heads @ Wproj[rows]. The host sums the 4 per-batch partials and adds
bproj.

All tensors are bf16 in DRAM/SBUF (f32 PSUM accumulation). The cost model
charges a matmul its OUTPUT free size per contraction tile, so attV runs
"transposed": exp tiles are the stationary operand and V the moving one,
accumulating [128 queries, 64 dims + 1 denom] per query-tile (free 65/tile
instead of 1024).  That halves attV's PE rows vs the [dims, queries]
layout and lands the softmax denominator in a PSUM *column*, so
normalization is a DVE reciprocal + per-partition tensor_scalar multiply
(no PE broadcast matmuls).  The normalized [query, dim] tiles for heads
0/1 of a query half pair up into [128,128] blocks that a PE
identity-transpose flips into the [dims, tokens] oT layout the projection
needs (h2 rides with a junk half).  Transposes and all non-attention
matmuls (V staging, heads 1-2 QK^T, the projection) are "filler units"
emitted inside the exp-bound attention j-loops so the PE chews them while
waiting on exp.  Pass order is head-major with h2 last; proj of the first
query half runs as filler in the last pass, proj of the second half is
the tail.
"""

import os
import sys

import numpy as np
import ml_dtypes

try:
    import concourse.bass as bass
except ImportError:  # harness runs from a bare directory
    sys.path.insert(0, "/opt/trn_rl_repo")
    import concourse.bass as bass

import concourse.tile as tile
from concourse import bacc, mybir
from concourse.bass_utils import run_bass_kernel_spmd
from concourse.masks import make_identity

F32 = mybir.dt.float32
BF16 = mybir.dt.bfloat16
AF = mybir.ActivationFunctionType
NPBF16 = ml_dtypes.bfloat16

NH = 12          # total heads
D = 64           # head dim
S = 2048         # sequence length
NI = 768         # model dim
NB = 2           # batch
NCORES = 8
HPC = 3          # heads per core
CPB = NCORES // NB   # cores per batch
KC = NI // 128   # contraction chunks for the 768 dim
NT = S // 128    # 128-row tiles along tokens/keys
HB = S // 2      # 1024: half the token/query axis
NQT = HB // 128  # 8 query tiles per half
SCALE = float(1.0 / np.sqrt(NI / NH))  # 1/8

# filled by kernel() for test.py to report
last_results = None

_cache = {}


def _build_nc(has_bias: bool):
    nc = bacc.Bacc("TRN2", target_bir_lowering=False, debug=False,
                   num_devices=NCORES)

    xT_d = nc.dram_tensor("xT", [NI, S], BF16, kind="ExternalInput")
    wqk_d = nc.dram_tensor("wqk", [NI, HPC * 128], BF16, kind="ExternalInput")
    # wv padded to 256 cols so each DMA descriptor is 512B (full-rate)
    wv_d = nc.dram_tensor("wv", [NI, 256], BF16, kind="ExternalInput")
    wp_d = nc.dram_tensor("wp", [HPC * D, NI], BF16, kind="ExternalInput")
    if has_bias:
        # cols 2h = bq_h, 2h+1 = bk_h (64 rows each); bv packed per-head
        bqk_d = nc.dram_tensor("bqk", [D, 2 * HPC], F32, kind="ExternalInput")
        bv_d = nc.dram_tensor("bv", [HPC * D], F32, kind="ExternalInput")
    out_d = nc.dram_tensor("out", [S, NI], BF16, kind="ExternalOutput")

    with tile.TileContext(nc) as tc:
        with (
            tc.tile_pool(name="const", bufs=1) as constp,
            tc.tile_pool(name="expp", bufs=6) as expp,
            tc.tile_pool(name="opool", bufs=1) as opool,
            tc.tile_pool(name="rwork", bufs=2) as rwork,
            # PSUM (8 banks x 2KB): tag A = 2 slots x 2 banks (QK^T h0 in
            # the load phase, then the 2-deep scores rotation, then proj
            # tail), tag B = 1 slot x 2 banks (V group 0, then the per-pass
            # [q, dims] attV accumulator), tag C/D = 1 bank each (in-pass
            # filler units and oT transposes).
            tc.tile_pool(name="ps", bufs=2, space="PSUM") as psp,
            tc.tile_pool(name="dramp", bufs=2, space="DRAM") as dramp,
        ):
            # ---- SBUF tensors ----
            xT = constp.tile([128, KC, S], BF16, tag="xT")
            wqk = constp.tile([128, KC, HPC * 128], BF16, tag="wqk")
            wv = constp.tile([128, KC, 256], BF16, tag="wv")
            wp01 = constp.tile([128, NI], BF16, tag="wp01")
            wp2 = constp.tile([D, NI], BF16, tag="wp2")
            qq = [constp.tile([D, S], BF16, tag=f"qq{h}", name=f"qq{h}")
                  for h in range(HPC)]
            kk = [constp.tile([D, S], BF16, tag=f"kk{h}", name=f"kk{h}")
                  for h in range(HPC)]
            # vaug layout: [128, h, NT*65]; per key-chunk j the slab
            # [:, h, 65j : 65j+65] is [V_h(chunk j) | ones].
            vaug = constp.tile([128, HPC, NT * 65], BF16, tag="vaug")
            oT01 = opool.tile([128, S], BF16, tag="oT01")
            oT2 = opool.tile([D, S], BF16, tag="oT2")
            # normalized [query, dim] staging per query half: heads 0/1
            # pack into cols 0:64 / 64:128 per 128-query tile; head 2 uses
            # its own slab with a junk upper half (never read after the
            # transpose).
            sb01 = [constp.tile([128, NQT, 128], BF16, tag=f"sb01_{qh}",
                                name=f"sb01_{qh}") for qh in range(2)]
            sb2 = [constp.tile([128, NQT, 128], BF16, tag=f"sb2_{qh}",
                               name=f"sb2_{qh}") for qh in range(2)]

            # ---- input DMAs. The first exp only needs Q/K of head 0 for
            # query half 0, i.e. xT columns 0:1024 over all 6 contraction
            # chunks plus the h0 wqk slabs. SP queue: xT half 0 with wv
            # interleaved (the V staging follows right behind the QK load),
            # then xT half 1, then the remaining weights. ACT queue (idle
            # until the first exp): the small h0 QK weight slabs.
            xT_src = xT_d[:].rearrange("(c p) s -> p c s", p=128)
            wqk_src = wqk_d[:].rearrange("(c p) m -> p c m", p=128)
            wv_src = wv_d[:].rearrange("(c p) m -> p c m", p=128)
            if has_bias:
                bqk = constp.tile([D, 2 * HPC], F32, tag="bqk")
                nc.sync.dma_start(out=bqk, in_=bqk_d[:])
                bvb = constp.tile([128, HPC * D], F32, tag="bvb")
                bv_ap = bv_d[:]
                bv_bcast = bass.AP(
                    tensor=bv_ap.tensor, offset=bv_ap.offset,
                    ap=[[0, 128]] + [list(p) for p in bv_ap.ap])
                nc.sync.dma_start(out=bvb, in_=bv_bcast)
                bvb3 = bvb.rearrange("p (h d) -> p h d", d=D)
            # Every dma_start costs ~630ns of serial HWDGE time, so the
            # gating path uses few, coarse descriptors: the h0 QK slabs in
            # one, xT half 0 in four (finer toward the gating chunk so the
            # per-chunk QK^T matmuls overlap the stream), wv in one.
            nc.sync.dma_start(out=wqk[:, :, 0:128], in_=wqk_src[:, :, 0:128])
            nc.sync.dma_start(out=xT[:, 0:3, 0:HB], in_=xT_src[:, 0:3, 0:HB])
            nc.sync.dma_start(out=xT[:, 3:5, 0:HB], in_=xT_src[:, 3:5, 0:HB])
            nc.sync.dma_start(out=xT[:, 5, 0:512], in_=xT_src[:, 5, 0:512])
            nc.sync.dma_start(out=xT[:, 5, 512:HB], in_=xT_src[:, 5, 512:HB])
            nc.sync.dma_start(out=wv[:, :, :], in_=wv_src[:, :, :])
            nc.sync.dma_start(out=xT[:, 0:3, HB:S], in_=xT_src[:, 0:3, HB:S])
            nc.sync.dma_start(out=xT[:, 3:6, HB:S], in_=xT_src[:, 3:6, HB:S])
            nc.sync.dma_start(out=wqk[:, :, 128:HPC * 128],
                              in_=wqk_src[:, :, 128:HPC * 128])
            nc.sync.dma_start(out=wp01, in_=wp_d[0:128, :])
            nc.sync.dma_start(out=wp2, in_=wp_d[128:HPC * D, :])

            # ---- ACT warmup: trigger the Exp table load at t=0 ----
            warm_in = constp.tile([1, 2], F32, tag="warm_in")
            warm_out = constp.tile([1, 2], F32, tag="warm_out")
            nc.vector.memset(warm_in, 0.0)
            nc.scalar.activation(warm_out, warm_in, AF.Exp, scale=1.0)

            # PE-warmup scratch first in the DVE queue so warm matmuls
            # start immediately
            scr = constp.tile([128, 512], BF16, tag="scr")
            nc.vector.memset(scr, 0.0)

            # identity for the PE transposes (gpsimd builds it off the
            # critical path)
            ident = constp.tile([128, 128], BF16, tag="ident")
            make_identity(nc, ident)

            # junk halves of the h2 staging slabs must still be defined
            # bits for the transpose reads
            for qh in range(2):
                nc.gpsimd.memset(sb2[qh][:, :, D:128], 0.0)

            # ones columns of vaug
            ones_sb = constp.tile([128, NT, 1], BF16, tag="ones")
            nc.vector.memset(ones_sb, 1.0)
            v4 = vaug.rearrange("p h (t c) -> p h t c", c=65)
            for h in range(HPC):
                nc.vector.tensor_copy(v4[:, h, :, 64:65], ones_sb)

            # ---- copy helpers (PSUM sources: DVE or ACT only --
            # GPSIMD cannot access PSUM) ----
            def qk_copy(h, sl, src, k_eng="v"):
                # src [128, len] psum (q rows 0:64, k rows 64:128); the k
                # copy shifts partitions 64-127 down to 0-63
                if has_bias:
                    nc.vector.tensor_scalar_add(
                        qq[h][:, sl], src[0:D, :], bqk[:, 2 * h:2 * h + 1])
                    nc.vector.tensor_scalar_add(
                        kk[h][:, sl], src[D:128, :],
                        bqk[:, 2 * h + 1:2 * h + 2])
                    return
                nc.vector.tensor_copy(qq[h][:, sl], src[0:D, :])
                if k_eng == "a":
                    nc.scalar.copy(kk[h][:, sl], src[D:128, :])
                else:
                    nc.vector.tensor_copy(kk[h][:, sl], src[D:128, :])

            def v_copy(pv, ti, t):
                # pv [128, n, 256] psum; one fused copy into all 3 heads'
                # vaug slabs
                dst = v4[:, :, t, 0:D]
                src = pv[:, ti, 0:HPC * D].rearrange("p (h d) -> p h d", d=D)
                if has_bias:
                    nc.vector.tensor_add(dst, src, bvb3)
                else:
                    nc.vector.tensor_copy(dst, src)

            # ---- PE p-state warmup into the bank the first real
            # start=True matmul will clear ----
            qktt = psp.tile([128, HB], F32, tag="A", name="qkt0")
            for _ in range(6):
                nc.tensor.matmul(qktt[:, 0:512], scr[:, 0:128], scr,
                                 start=True, stop=True,
                                 skip_group_check=True)

            # ---- load phase: QK^T(h0, query half 0) tracks the xT chunk
            # stream; half 1 runs as qkt_piece fillers inside pass 0 once
            # the xT half-1 chunks land. The V matmuls (wv rides between
            # the xT half-0 chunks) are emitted between the scores
            # prologue and the j-loop ----
            pv0 = psp.tile([128, 4, 256], F32, tag="B", name="pv0", bufs=1)
            for c in range(KC):
                for n in range(2):
                    nc.tensor.matmul(
                        qktt[:, n * 512:(n + 1) * 512],
                        wqk[:, c, 0:128], xT[:, c, n * 512:(n + 1) * 512],
                        start=(c == 0), stop=(c == KC - 1),
                        skip_group_check=True)
            # gate sc(0)/sc(1): q copies on DVE; k cols 0:512 (all the
            # early scores need) on the idle ACT, k cols 512:1024 on DVE
            # behind q so exp(0) isn't queued behind it
            nc.vector.tensor_copy(qq[0][:, 0:512], qktt[0:D, 0:512])
            nc.vector.tensor_copy(qq[0][:, 512:HB], qktt[0:D, 512:HB])
            nc.scalar.copy(kk[0][:, 0:512], qktt[D:128, 0:512])
            nc.vector.tensor_copy(kk[0][:, 512:HB], qktt[D:128, 512:HB])

            def pv_mms(pv, base, pair=None):
                tis = (range(pv.shape[1]) if pair is None
                       else (2 * pair, 2 * pair + 1))
                for c in range(KC):
                    for ti in tis:
                        t = base + ti
                        nc.tensor.matmul(
                            pv[:, ti, 0:HPC * D],
                            xT[:, c, t * 128:(t + 1) * 128],
                            wv[:, c, 0:HPC * D],
                            start=(c == 0 and ti % 2 == 0),
                            stop=(c == KC - 1), skip_group_check=True)

            def p0_preloop():
                # V tiles 0-3: matmuls on the PE while the h0 copies and
                # the first exps run; bank-pair order so v_copy(0) (which
                # gates attV j=0) lands early. Tiles 4-7 ride as slot-0/1
                # fillers, 8-15 as later fillers.
                pv_mms(pv0, 0, pair=0)
                v_copy(pv0, 0, 0)
                v_copy(pv0, 1, 1)
                pv_mms(pv0, 0, pair=1)
                v_copy(pv0, 2, 2)
                v_copy(pv0, 3, 3)
                if has_bias:
                    nc.vector.tensor_scalar_add(
                        qq[0][:, 0:HB], qq[0][:, 0:HB], bqk[:, 0:1])
                    nc.vector.tensor_scalar_add(
                        kk[0][:, 0:HB], kk[0][:, 0:HB], bqk[:, 1:2])

            def pv_unit(base, tag):
                def emit():
                    pv = psp.tile([128, 2, 256], F32, tag=tag,
                                  name=f"pv{base}", bufs=1)
                    pv_mms(pv, base)
                    for ti in range(pv.shape[1]):
                        v_copy(pv, ti, base + ti)
                return emit

            # ---- filler units: each fits a single PSUM bank, and
            # alternates between the C and D slots so one unit's drain
            # copy overlaps the next unit's matmuls ----
            def v_unit(t, tag):
                def emit():
                    pvu = psp.tile([128, 1, 256], F32, tag=tag, bufs=1,
                                   name=f"vu{t}")
                    for c in range(KC):
                        nc.tensor.matmul(
                            pvu[:, 0, 0:HPC * D],
                            xT[:, c, t * 128:(t + 1) * 128],
                            wv[:, c, 0:HPC * D],
                            start=(c == 0), stop=(c == KC - 1))
                    v_copy(pvu, 0, t)
                return emit

            def qkt_piece(h, half, n, tag):
                # three emissions (consecutive filler slots) so the 6-chunk
                # matmul burst doesn't starve the exp stream
                state = {}
                sl = slice(half * HB + n * 512, half * HB + (n + 1) * 512)

                def mm_range(c0, c1):
                    for c in range(c0, c1):
                        nc.tensor.matmul(
                            state["pc"], wqk[:, c, h * 128:(h + 1) * 128],
                            xT[:, c, sl],
                            start=(c == 0), stop=(c == KC - 1))

                def emit_a():
                    state["pc"] = psp.tile([128, 512], F32, tag=tag, bufs=1,
                                           name=f"qp{h}_{half}_{n}")
                    mm_range(0, 2)

                def emit_b():
                    mm_range(2, 4)

                def emit_c():
                    mm_range(4, KC)
                    qk_copy(h, sl, state["pc"])
                return emit_a, emit_b, emit_c

            # ---- oT transpose units: one [128,128] normalized [q, d]
            # block -> PE identity transpose -> [d, q] drain into oT ----
            def t_unit(h01, qh, qts, tag, eng="v"):
                # h01: True = heads 0/1 pair into oT01, False = head 2
                src = sb01[qh] if h01 else sb2[qh]

                def emit():
                    for i, qt in enumerate(qts):
                        tp = psp.tile([128, 128], BF16,
                                      tag=(tag if i % 2 == 0 else
                                           ("D" if tag == "C" else "C")),
                                      bufs=1, name=f"tp{int(h01)}{qh}{qt}")
                        nc.tensor.transpose(tp, src[:, qt, :], ident)
                        dst_sl = slice(qh * HB + qt * 128,
                                       qh * HB + (qt + 1) * 128)
                        if h01:
                            dst = oT01[:, dst_sl]
                            srcp = tp
                        else:
                            dst = oT2[:, dst_sl]
                            srcp = tp[0:D, :]
                        if eng == "a":
                            nc.scalar.copy(dst, srcp)
                        else:
                            nc.vector.tensor_copy(dst, srcp)
                return emit

            # ---- projection ----
            out_dst = out_d[:].rearrange("(t p) o -> t p o", p=128)
            ostage = {}

            def ost_for(t):
                if t not in ostage:
                    ostage[t] = rwork.tile([128, NI], BF16, tag="ostage",
                                           bufs=6, name=f"ost{t}")
                return ostage[t]

            def proj_piece(t, r, tag, eng):
                # one output region (r=0: cols 0:512, r=1: 512:768) of proj
                # tile t through a single-bank slot
                def emit():
                    n0, n1 = ((0, 512), (512, NI))[r]
                    pp = psp.tile([128, 512], F32, tag=tag, bufs=1,
                                  name=f"pp{t}_{r}")
                    o01 = oT01[:, t * 128:(t + 1) * 128]
                    o2 = oT2[:, t * 128:(t + 1) * 128]
                    nc.tensor.matmul(pp[:, 0:n1 - n0], o01, wp01[:, n0:n1],
                                     start=True, stop=False)
                    nc.tensor.matmul(pp[:, 0:n1 - n0], o2, wp2[:, n0:n1],
                                     start=False, stop=True)
                    ost = ost_for(t)
                    if eng == "a":
                        nc.scalar.copy(ost[:, n0:n1], pp[:, 0:n1 - n0])
                    else:
                        nc.vector.tensor_copy(ost[:, n0:n1], pp[:, 0:n1 - n0])
                    nc.sync.dma_start(out=out_dst[t][:, n0:n1],
                                      in_=ost[:, n0:n1])
                return emit

            def proj_mm1(t, tag):
                pp = psp.tile([128, HB], F32, tag=tag,
                              bufs=(None if tag == "A" else 1),
                              name=f"pp{t}")
                o01 = oT01[:, t * 128:(t + 1) * 128]
                for n0, n1 in ((0, 512), (512, NI)):
                    nc.tensor.matmul(pp[:, n0:n1], o01, wp01[:, n0:n1],
                                     start=True, stop=False)
                return pp

            def proj_mm2(t, pp, eng):
                o2 = oT2[:, t * 128:(t + 1) * 128]
                for n0, n1 in ((0, 512), (512, NI)):
                    nc.tensor.matmul(pp[:, n0:n1], o2, wp2[:, n0:n1],
                                     start=False, stop=True)
                ost = ost_for(t)
                if eng == "a":
                    nc.scalar.copy(ost, pp[:, 0:NI])
                else:
                    nc.vector.tensor_copy(ost, pp[:, 0:NI])
                nc.sync.dma_start(out=out_dst[t], in_=ost)

            def proj_unit(t, tag, eng):
                def emit():
                    proj_mm2(t, proj_mm1(t, tag), eng)
                return emit

            # ---- attention pass ----
            def sc_tile(h, qh, j):
                sc = psp.tile([128, HB], F32, tag="A", name="sc")
                klhs = kk[h][:, j * 128:(j + 1) * 128]
                for n in range(2):
                    sl = slice(qh * HB + n * 512, qh * HB + (n + 1) * 512)
                    nc.tensor.matmul(
                        sc[:, n * 512:(n + 1) * 512], klhs, qq[h][:, sl])
                return sc

            prologues = {}

            def attention_pass(h, qh, filler, last=False, pre_loop=None,
                               nxt=None):
                # acc2 [128 queries, 8 qt, 128]: per query tile, cols 0:64
                # hold out_h[q, d] unnormalized and col 64 the softmax
                # denominator (ones column of vaug); cols 65:128 are pad
                # so each qt slab is 512B and 4 slabs fill a PSUM bank.
                acc2 = psp.tile([128, NQT, 128], F32, tag="B",
                                name=f"acc{h}{qh}", bufs=1)

                sc_q = prologues.pop((h, qh), None)
                if sc_q is None:
                    sc_q = [sc_tile(h, qh, 0)]
                sc_q.append(sc_tile(h, qh, 1))
                if pre_loop is not None:
                    pre_loop()
                fq = list(filler)
                for j in range(NT):
                    # prefire the next pass's first score tile ahead of the
                    # final exp/attV so the next exp(0) has zero gap: its A
                    # slot is free as soon as exp(j=14) has read it
                    if j == NT - 1 and nxt is not None:
                        prologues[nxt] = [sc_tile(*nxt, 0)]
                    ex = expp.tile([128, HB], BF16, tag="exp")
                    nc.scalar.activation(ex, sc_q.pop(0), AF.Exp,
                                         scale=SCALE)
                    if j + 2 < NT:
                        sc_q.append(sc_tile(h, qh, j + 2))
                    # filler between the scores matmul and attV so the PE
                    # has work while exp(j) is still running
                    for _ in range(2):
                        if fq and fq[0][0] <= j:
                            fq.pop(0)[1]()
                        else:
                            break
                    vslab = vaug[:, h, j * 65:j * 65 + 65]
                    for qt in range(NQT):
                        nc.tensor.matmul(
                            acc2[:, qt, 0:65],
                            ex[:, qt * 128:(qt + 1) * 128], vslab,
                            start=(j == 0 and qt % 4 == 0),
                            stop=(j == NT - 1), skip_group_check=True)
                for _, f in fq:  # leftovers (shouldn't happen)
                    f()
                return acc2

            def norm2(h, qh, acc2):
                # acc2 col 64 holds the per-query denominator: DVE
                # reciprocal, then scale cols 0:64 into the bf16 [q, d]
                # staging slab (single-op psum release per qt slice).
                rcp = rwork.tile([128, NQT, 1], F32, tag="rcp", bufs=2)
                nc.vector.reciprocal(rcp, acc2[:, :, 64:65])
                dst = sb01[qh] if h < 2 else sb2[qh]
                col = h * D if h < 2 else 0
                for qt in range(NQT):
                    nc.vector.tensor_scalar_mul(
                        dst[:, qt, col:col + D], acc2[:, qt, 0:D],
                        rcp[:, qt])

            # ---- pass schedule (head-major, h2 last) ----
            # p0 (h0,q0): k(h0) half1 copy + V tiles 8-15
            # p1 (h0,q1): QK^T(h1) pieces
            # p2 (h1,q0): QK^T(h2) pieces 0-1
            # p3 (h1,q1): QK^T(h2) pieces 2-3 + oT01 transpose (q-half 0)
            # p4 (h2,q0): oT01 transpose (q-half 1)
            # p5 (h2,q1): oT2 transpose (q-half 0) + proj of query half 0
            # tail: oT2 transpose (q-half 1) + proj of query half 1
            cd = ("C", "D")

            def pieces(h, entries, j0, dj):
                out = []
                for i, (hf, n) in enumerate(entries):
                    for k, e in enumerate(qkt_piece(h, hf, n, cd[i % 2])):
                        out.append((j0 + dj * i + k, e))
                return out

            # p0: V tiles 4-7 at slots 0/1, then the h0 half-1 QK pieces
            # (keys 1024:1536 must land before sc(8) is emitted at j=6,
            # keys 1536:2048 before sc(12) at j=10 -- emitting them later
            # would deadlock the in-order PE queue), V tiles 8-15 spread
            # so tile t lands before attV consumes it at j=t.
            p0 = ([(0, pv_unit(4, "C")), (1, pv_unit(6, "D"))]
                  + pieces(0, ((1, 0),), 3, 1)
                  + pieces(0, ((1, 1),), 6, 1)
                  + [(7 + i, v_unit(8 + i, cd[(i + 1) % 2]))
                     for i in range(8)])
            p0.sort(key=lambda x: x[0])
            p1 = pieces(1, ((0, 0), (0, 1), (1, 0), (1, 1)), 3, 3)
            p2 = pieces(2, ((0, 0), (0, 1)), 3, 5)
            p3 = pieces(2, ((1, 0), (1, 1)), 3, 5)
            p3 += [(13 + i, t_unit(True, 0, qts, cd[i % 2]))
                   for i, qts in enumerate(((0, 1, 2), (3, 4, 5), (6, 7)))]
            p4 = [(1 + 2 * i, t_unit(True, 1, qts, cd[i % 2]))
                  for i, qts in enumerate(((0, 1, 2), (3, 4, 5), (6, 7)))]
            p5 = [(i, t_unit(False, 0, qts, cd[i % 2]))
                  for i, qts in enumerate(((0, 1, 2), (3, 4, 5), (6, 7)))]
            # last proj tile of the q0 half is emitted in the tail so its
            # drain copies don't queue ahead of the tail reciprocal
            p5 += [(3 + i, proj_piece(i // 2, i % 2, cd[i % 2], "v"))
                   for i in range(14)]
            acc00 = attention_pass(0, 0, p0, pre_loop=p0_preloop,
                                   nxt=(0, 1))
            acc01 = attention_pass(0, 1, p1,
                                   pre_loop=lambda: norm2(0, 0, acc00),
                                   nxt=(1, 0))
            acc10 = attention_pass(1, 0, p2,
                                   pre_loop=lambda: norm2(0, 1, acc01),
                                   nxt=(1, 1))
            acc11 = attention_pass(1, 1, p3,
                                   pre_loop=lambda: norm2(1, 0, acc10),
                                   nxt=(2, 0))
            acc20 = attention_pass(2, 0, p4,
                                   pre_loop=lambda: norm2(1, 1, acc11),
                                   nxt=(2, 1))
            acc_last = attention_pass(2, 1, p5,
                                      pre_loop=lambda: norm2(2, 0, acc20),
                                      last=True)

            # ---- tail: normalize h2/q1, transpose it into oT2, and
            # project tiles 8-15. Each proj tile 8+qt only needs oT2
            # query tile qt, so the tail runs a per-qt pipeline:
            # scale (DVE) -> PE transpose -> drain (DVE) -> proj matmuls
            # (PE) -> copy split ACT/DVE -> output DMA. ACT is idle after
            # the last exp, so it takes the larger copy half.
            pp_pre = {t: proj_mm1(t, "A") for t in (8, 9)}
            rcp_t = rwork.tile([128, NQT, 1], F32, tag="rcp", bufs=2)
            nc.vector.reciprocal(rcp_t, acc_last[:, :, 64:65])
            proj_piece(7, 0, "C", "a")()
            proj_piece(7, 1, "D", "a")()
            # DVE burst: normalize all 8 qt (also frees the acc banks),
            # then a per-qt pipeline: PE transpose -> drain -> proj mm ->
            # copy halves split across ACT and DVE -> full-tile DMA.
            for qt in range(NQT):
                nc.vector.tensor_scalar_mul(
                    sb2[1][:, qt, 0:D], acc_last[:, qt, 0:D], rcp_t[:, qt])
            for qt in range(NQT):
                dr_a = qt % 2 == 0
                tp = psp.tile([128, 128], BF16, tag=cd[qt % 2], bufs=1,
                              name=f"tpt{qt}")
                nc.tensor.transpose(tp, sb2[1][:, qt, :], ident)
                o2sl = oT2[:, HB + qt * 128:HB + (qt + 1) * 128]
                if dr_a:
                    nc.scalar.copy(o2sl, tp[0:D, :])
                else:
                    nc.vector.tensor_copy(o2sl, tp[0:D, :])
                t8 = 8 + qt
                pp = (pp_pre[t8] if t8 in pp_pre else
                      proj_mm1(t8, ("B", "A", "A", "B", "A", "A")[qt - 2]))
                o2 = oT2[:, t8 * 128:(t8 + 1) * 128]
                for n0, n1 in ((0, 512), (512, NI)):
                    nc.tensor.matmul(pp[:, n0:n1], o2, wp2[:, n0:n1],
                                     start=False, stop=True)
                ost = ost_for(t8)
                if dr_a:
                    nc.vector.tensor_copy(ost[:, 0:512], pp[:, 0:512])
                    nc.scalar.copy(ost[:, 512:NI], pp[:, 512:NI])
                else:
                    nc.scalar.copy(ost[:, 0:512], pp[:, 0:512])
                    nc.vector.tensor_copy(ost[:, 512:NI], pp[:, 512:NI])
                nc.sync.dma_start(out=out_dst[t8], in_=ost)

    nc.compile()
    return nc


def _get_nc(has_bias: bool):
    if has_bias not in _cache:
        _cache[has_bias] = _build_nc(has_bias)
    return _cache[has_bias]


def kernel(inp, Wqkv, bqkv, Wproj, bproj):
    global last_results
    inp = np.ascontiguousarray(np.asarray(inp, dtype=np.float32))
    Wqkv = np.asarray(Wqkv, dtype=np.float32)
    bqkv = np.asarray(bqkv, dtype=np.float32)
    Wproj = np.asarray(Wproj, dtype=np.float32)
    bproj = np.asarray(bproj, dtype=np.float32)
    assert inp.shape == (NB, S, NI), inp.shape

    has_bias = bool(np.any(bqkv))
    nc = _get_nc(has_bias)

    xTs = [np.ascontiguousarray(inp[b].T).astype(NPBF16) for b in range(NB)]

    in_maps = []
    for core in range(NCORES):
        b = core // CPB
        heads = [(core % CPB) * HPC + i for i in range(HPC)]
        wqk = np.empty((NI, HPC * 128), np.float32)
        wvm = np.zeros((NI, 256), np.float32)
        wp = np.empty((HPC * D, NI), np.float32)
        for i, h in enumerate(heads):
            base = h * 3 * D
            wqk[:, i * 128:i * 128 + D] = Wqkv[:, base:base + D]
            wqk[:, i * 128 + D:(i + 1) * 128] = Wqkv[:, base + D:base + 2 * D]
            wvm[:, i * D:(i + 1) * D] = Wqkv[:, base + 2 * D:base + 3 * D]
            wp[i * D:(i + 1) * D, :] = Wproj[h * D:(h + 1) * D, :]
        m = {"xT": xTs[b], "wqk": wqk.astype(NPBF16),
             "wv": wvm.astype(NPBF16), "wp": wp.astype(NPBF16)}
        if has_bias:
            bqk = np.empty((D, 2 * HPC), np.float32)
            bv = np.empty((HPC * D,), np.float32)
            for i, h in enumerate(heads):
                base = h * 3 * D
                bqk[:, 2 * i] = bqkv[base:base + D]
                bqk[:, 2 * i + 1] = bqkv[base + D:base + 2 * D]
                bv[i * D:(i + 1) * D] = bqkv[base + 2 * D:base + 3 * D]
            m["bqk"] = bqk
            m["bv"] = bv
        in_maps.append(m)

    res = run_bass_kernel_spmd(nc, in_maps, core_ids=list(range(NCORES)))
    last_results = res

    out = np.zeros((NB, S, NI), np.float32)
    for core in range(NCORES):
        out[core // CPB] += np.asarray(res.results[core]["out"],
                                       dtype=np.float32)
    out += bproj
    return out


# revision 18
# speedup vs baseline: 1.0404x; 1.0218x over previous
"""Trainium2 Bass kernel for nn_MultiHeadAttention_10960756539999.

MHA: inp [2, 2048, 768], 12 heads, head_dim 64, Wqkv [768, 2304] (per-head
192-col slabs laid out [Q|K|V]), Wproj [768, 768].

Sharding: 24 (batch, head) pairs -> 3 heads per core; cores 0-3 take batch 0,
cores 4-7 take batch 1. Each core computes QKV^T for its heads from x^T,
attention fully on-chip (softmax over the free axis of scores^T, no max
subtraction -- scores are ~N(0,1)), and a row-sharded partial projection
out_heads @ Wproj[rows]. The host sums the 4 per-batch partials and adds
bproj.

All tensors are bf16 in DRAM/SBUF (f32 PSUM accumulation). The cost model
charges a matmul its OUTPUT free size per contraction tile, so attV runs
"transposed": exp tiles are the stationary operand and V the moving one,
accumulating [128 queries, 64 dims + 1 denom] per query-tile (free 65/tile
instead of 1024).  That halves attV's PE rows vs the [dims, queries]
layout and lands the softmax denominator in a PSUM *column*, so
normalization is a DVE reciprocal + per-partition tensor_scalar multiply
(no PE broadcast matmuls).  The normalized [query, dim] tiles for heads
0/1 of a query half pair up into [128,128] blocks that a PE
identity-transpose flips into the [dims, tokens] oT layout the projection
needs (h2 rides with a junk half).  Transposes and all non-attention
matmuls (V staging, heads 1-2 QK^T, the projection) are "filler units"
emitted inside the exp-bound attention j-loops so the PE chews them while
waiting on exp.  Pass order is head-major with h2 last; proj of the first
query half runs as filler in the last pass, proj of the second half is
the tail.
"""

import os
import sys

import numpy as np
import ml_dtypes

try:
    import concourse.bass as bass
except ImportError:  # harness runs from a bare directory
    sys.path.insert(0, "/opt/trn_rl_repo")
    import concourse.bass as bass

import concourse.tile as tile
from concourse import bacc, mybir
from concourse.bass_utils import run_bass_kernel_spmd
from concourse.masks import make_identity

F32 = mybir.dt.float32
BF16 = mybir.dt.bfloat16
AF = mybir.ActivationFunctionType
NPBF16 = ml_dtypes.bfloat16

NH = 12          # total heads
D = 64           # head dim
S = 2048         # sequence length
NI = 768         # model dim
NB = 2           # batch
NCORES = 8
HPC = 3          # heads per core
CPB = NCORES // NB   # cores per batch
KC = NI // 128   # contraction chunks for the 768 dim
NT = S // 128    # 128-row tiles along tokens/keys
HB = S // 2      # 1024: half the token/query axis
NQT = HB // 128  # 8 query tiles per half
SCALE = float(1.0 / np.sqrt(NI / NH))  # 1/8

# filled by kernel() for test.py to report
last_results = None

_cache = {}


def _build_nc(has_bias: bool):
    nc = bacc.Bacc("TRN2", target_bir_lowering=False, debug=False,
                   num_devices=NCORES)

    xT_d = nc.dram_tensor("xT", [NI, S], BF16, kind="ExternalInput")
    wqk_d = nc.dram_tensor("wqk", [NI, HPC * 128], BF16, kind="ExternalInput")
    # wv padded to 256 cols so each DMA descriptor is 512B (full-rate)
    wv_d = nc.dram_tensor("wv", [NI, 256], BF16, kind="ExternalInput")
    wp_d = nc.dram_tensor("wp", [HPC * D, NI], BF16, kind="ExternalInput")
    if has_bias:
        # cols 2h = bq_h, 2h+1 = bk_h (64 rows each); bv packed per-head
        bqk_d = nc.dram_tensor("bqk", [D, 2 * HPC], F32, kind="ExternalInput")
        bv_d = nc.dram_tensor("bv", [HPC * D], F32, kind="ExternalInput")
    out_d = nc.dram_tensor("out", [S, NI], BF16, kind="ExternalOutput")

    with tile.TileContext(nc) as tc:
        with (
            tc.tile_pool(name="const", bufs=1) as constp,
            tc.tile_pool(name="expp", bufs=6) as expp,
            tc.tile_pool(name="opool", bufs=1) as opool,
            tc.tile_pool(name="rwork", bufs=2) as rwork,
            # PSUM (8 banks x 2KB): tag A = 2 slots x 2 banks (QK^T h0 in
            # the load phase, then the 2-deep scores rotation, then proj
            # tail), tag B = 1 slot x 2 banks (V group 0, then the per-pass
            # [q, dims] attV accumulator), tag C/D = 1 bank each (in-pass
            # filler units and oT transposes).
            tc.tile_pool(name="ps", bufs=2, space="PSUM") as psp,
            tc.tile_pool(name="dramp", bufs=2, space="DRAM") as dramp,
        ):
            # ---- SBUF tensors ----
            xT = constp.tile([128, KC, S], BF16, tag="xT")
            wqk = constp.tile([128, KC, HPC * 128], BF16, tag="wqk")
            wv = constp.tile([128, KC, 256], BF16, tag="wv")
            wp01 = constp.tile([128, NI], BF16, tag="wp01")
            wp2 = constp.tile([D, NI], BF16, tag="wp2")
            qq = [constp.tile([D, S], BF16, tag=f"qq{h}", name=f"qq{h}")
                  for h in range(HPC)]
            kk = [constp.tile([D, S], BF16, tag=f"kk{h}", name=f"kk{h}")
                  for h in range(HPC)]
            # vaug layout: [128, h, NT*65]; per key-chunk j the slab
            # [:, h, 65j : 65j+65] is [V_h(chunk j) | ones].
            vaug = constp.tile([128, HPC, NT * 65], BF16, tag="vaug")
            oT01 = opool.tile([128, S], BF16, tag="oT01")
            oT2 = opool.tile([D, S], BF16, tag="oT2")
            # normalized [query, dim] staging per query half: heads 0/1
            # pack into cols 0:64 / 64:128 per 128-query tile; head 2 uses
            # its own slab with a junk upper half (never read after the
            # transpose).
            sb01 = [constp.tile([128, NQT, 128], BF16, tag=f"sb01_{qh}",
                                name=f"sb01_{qh}") for qh in range(2)]
            sb2 = [constp.tile([128, NQT, 128], BF16, tag=f"sb2_{qh}",
                               name=f"sb2_{qh}") for qh in range(2)]

            # ---- input DMAs. The first exp only needs Q/K of head 0 for
            # query half 0, i.e. xT columns 0:1024 over all 6 contraction
            # chunks plus the h0 wqk slabs. SP queue: xT half 0 with wv
            # interleaved (the V staging follows right behind the QK load),
            # then xT half 1, then the remaining weights. ACT queue (idle
            # until the first exp): the small h0 QK weight slabs.
            xT_src = xT_d[:].rearrange("(c p) s -> p c s", p=128)
            wqk_src = wqk_d[:].rearrange("(c p) m -> p c m", p=128)
            wv_src = wv_d[:].rearrange("(c p) m -> p c m", p=128)
            if has_bias:
                bqk = constp.tile([D, 2 * HPC], F32, tag="bqk")
                nc.sync.dma_start(out=bqk, in_=bqk_d[:])
                bvb = constp.tile([128, HPC * D], F32, tag="bvb")
                bv_ap = bv_d[:]
                bv_bcast = bass.AP(
                    tensor=bv_ap.tensor, offset=bv_ap.offset,
                    ap=[[0, 128]] + [list(p) for p in bv_ap.ap])
                nc.sync.dma_start(out=bvb, in_=bv_bcast)
                bvb3 = bvb.rearrange("p (h d) -> p h d", d=D)
            # Every dma_start costs ~630ns of serial HWDGE time, so the
            # gating path uses few, coarse descriptors: the h0 QK slabs in
            # one, xT half 0 in four (finer toward the gating chunk so the
            # per-chunk QK^T matmuls overlap the stream), wv in one.
            nc.sync.dma_start(out=wqk[:, :, 0:128], in_=wqk_src[:, :, 0:128])
            nc.sync.dma_start(out=xT[:, 0:3, 0:HB], in_=xT_src[:, 0:3, 0:HB])
            nc.sync.dma_start(out=xT[:, 3:5, 0:HB], in_=xT_src[:, 3:5, 0:HB])
            nc.sync.dma_start(out=xT[:, 5, 0:512], in_=xT_src[:, 5, 0:512])
            nc.sync.dma_start(out=xT[:, 5, 512:HB], in_=xT_src[:, 5, 512:HB])
            nc.sync.dma_start(out=wv[:, :, :], in_=wv_src[:, :, :])
            nc.sync.dma_start(out=xT[:, 0:3, HB:S], in_=xT_src[:, 0:3, HB:S])
            nc.sync.dma_start(out=xT[:, 3:6, HB:S], in_=xT_src[:, 3:6, HB:S])
            nc.sync.dma_start(out=wqk[:, :, 128:HPC * 128],
                              in_=wqk_src[:, :, 128:HPC * 128])
            nc.sync.dma_start(out=wp01, in_=wp_d[0:128, :])
            nc.sync.dma_start(out=wp2, in_=wp_d[128:HPC * D, :])

            # ---- ACT warmup: trigger the Exp table load at t=0 ----
            warm_in = constp.tile([1, 2], F32, tag="warm_in")
            warm_out = constp.tile([1, 2], F32, tag="warm_out")
            nc.vector.memset(warm_in, 0.0)
            nc.scalar.activation(warm_out, warm_in, AF.Exp, scale=1.0)

            # PE-warmup scratch first in the DVE queue so warm matmuls
            # start immediately
            scr = constp.tile([128, 512], BF16, tag="scr")
            nc.vector.memset(scr, 0.0)

            # identity for the PE transposes (gpsimd builds it off the
            # critical path)
            ident = constp.tile([128, 128], BF16, tag="ident")
            make_identity(nc, ident)

            # junk halves of the h2 staging slabs must still be defined
            # bits for the transpose reads
            for qh in range(2):
                nc.gpsimd.memset(sb2[qh][:, :, D:128], 0.0)

            # ones columns of vaug
            ones_sb = constp.tile([128, NT, 1], BF16, tag="ones")
            nc.vector.memset(ones_sb, 1.0)
            v4 = vaug.rearrange("p h (t c) -> p h t c", c=65)
            for h in range(HPC):
                nc.vector.tensor_copy(v4[:, h, :, 64:65], ones_sb)

            # ---- copy helpers (PSUM sources: DVE or ACT only --
            # GPSIMD cannot access PSUM) ----
            def qk_copy(h, sl, src, k_eng="v"):
                # src [128, len] psum (q rows 0:64, k rows 64:128); the k
                # copy shifts partitions 64-127 down to 0-63
                if has_bias:
                    nc.vector.tensor_scalar_add(
                        qq[h][:, sl], src[0:D, :], bqk[:, 2 * h:2 * h + 1])
                    nc.vector.tensor_scalar_add(
                        kk[h][:, sl], src[D:128, :],
                        bqk[:, 2 * h + 1:2 * h + 2])
                    return
                nc.vector.tensor_copy(qq[h][:, sl], src[0:D, :])
                if k_eng == "a":
                    nc.scalar.copy(kk[h][:, sl], src[D:128, :])
                else:
                    nc.vector.tensor_copy(kk[h][:, sl], src[D:128, :])

            def v_copy(pv, ti, t):
                # pv [128, n, 256] psum; one fused copy into all 3 heads'
                # vaug slabs
                dst = v4[:, :, t, 0:D]
                src = pv[:, ti, 0:HPC * D].rearrange("p (h d) -> p h d", d=D)
                if has_bias:
                    nc.vector.tensor_add(dst, src, bvb3)
                else:
                    nc.vector.tensor_copy(dst, src)

            # ---- PE p-state warmup into the bank the first real
            # start=True matmul will clear ----
            qktt = psp.tile([128, HB], F32, tag="A", name="qkt0")
            for _ in range(6):
                nc.tensor.matmul(qktt[:, 0:512], scr[:, 0:128], scr,
                                 start=True, stop=True,
                                 skip_group_check=True)

            # ---- load phase: QK^T(h0, query half 0) tracks the xT chunk
            # stream; half 1 runs as qkt_piece fillers inside pass 0 once
            # the xT half-1 chunks land. The V matmuls (wv rides between
            # the xT half-0 chunks) are emitted between the scores
            # prologue and the j-loop ----
            pv0 = psp.tile([128, 4, 256], F32, tag="B", name="pv0", bufs=1)
            for c in range(KC):
                for n in range(2):
                    nc.tensor.matmul(
                        qktt[:, n * 512:(n + 1) * 512],
                        wqk[:, c, 0:128], xT[:, c, n * 512:(n + 1) * 512],
                        start=(c == 0), stop=(c == KC - 1),
                        skip_group_check=True)
            # gate sc(0)/sc(1): q copies on DVE; k cols 0:512 (all the
            # early scores need) on the idle ACT, k cols 512:1024 on DVE
            # behind q so exp(0) isn't queued behind it
            nc.vector.tensor_copy(qq[0][:, 0:512], qktt[0:D, 0:512])
            nc.vector.tensor_copy(qq[0][:, 512:HB], qktt[0:D, 512:HB])
            nc.scalar.copy(kk[0][:, 0:512], qktt[D:128, 0:512])
            nc.vector.tensor_copy(kk[0][:, 512:HB], qktt[D:128, 512:HB])

            def pv_mms(pv, base, pair=None):
                tis = (range(pv.shape[1]) if pair is None
                       else (2 * pair, 2 * pair + 1))
                for c in range(KC):
                    for ti in tis:
                        t = base + ti
                        nc.tensor.matmul(
                            pv[:, ti, 0:HPC * D],
                            xT[:, c, t * 128:(t + 1) * 128],
                            wv[:, c, 0:HPC * D],
                            start=(c == 0 and ti % 2 == 0),
                            stop=(c == KC - 1), skip_group_check=True)

            def p0_preloop():
                # V tiles 0-3: matmuls on the PE while the h0 copies and
                # the first exps run; bank-pair order so v_copy(0) (which
                # gates attV j=0) lands early. Tiles 4-7 ride as slot-0/1
                # fillers, 8-15 as later fillers.
                pv_mms(pv0, 0, pair=0)
                v_copy(pv0, 0, 0)
                v_copy(pv0, 1, 1)
                pv_mms(pv0, 0, pair=1)
                v_copy(pv0, 2, 2)
                v_copy(pv0, 3, 3)
                if has_bias:
                    nc.vector.tensor_scalar_add(
                        qq[0][:, 0:HB], qq[0][:, 0:HB], bqk[:, 0:1])
                    nc.vector.tensor_scalar_add(
                        kk[0][:, 0:HB], kk[0][:, 0:HB], bqk[:, 1:2])

            def pv_unit(base, tag):
                def emit():
                    pv = psp.tile([128, 2, 256], F32, tag=tag,
                                  name=f"pv{base}", bufs=1)
                    pv_mms(pv, base)
                    for ti in range(pv.shape[1]):
                        v_copy(pv, ti, base + ti)
                return emit

            # ---- filler units: each fits a single PSUM bank, and
            # alternates between the C and D slots so one unit's drain
            # copy overlaps the next unit's matmuls ----
            def v_unit(t, tag):
                def emit():
                    pvu = psp.tile([128, 1, 256], F32, tag=tag, bufs=1,
                                   name=f"vu{t}")
                    for c in range(KC):
                        nc.tensor.matmul(
                            pvu[:, 0, 0:HPC * D],
                            xT[:, c, t * 128:(t + 1) * 128],
                            wv[:, c, 0:HPC * D],
                            start=(c == 0), stop=(c == KC - 1))
                    v_copy(pvu, 0, t)
                return emit

            def qkt_piece(h, half, n, tag):
                # three emissions (consecutive filler slots) so the 6-chunk
                # matmul burst doesn't starve the exp stream
                state = {}
                sl = slice(half * HB + n * 512, half * HB + (n + 1) * 512)

                def mm_range(c0, c1):
                    for c in range(c0, c1):
                        nc.tensor.matmul(
                            state["pc"], wqk[:, c, h * 128:(h + 1) * 128],
                            xT[:, c, sl],
                            start=(c == 0), stop=(c == KC - 1))

                def emit_a():
                    state["pc"] = psp.tile([128, 512], F32, tag=tag, bufs=1,
                                           name=f"qp{h}_{half}_{n}")
                    mm_range(0, 2)

                def emit_b():
                    mm_range(2, 4)

                def emit_c():
                    mm_range(4, KC)
                    qk_copy(h, sl, state["pc"])
                return emit_a, emit_b, emit_c

            # ---- oT transpose units: one [128,128] normalized [q, d]
            # block -> PE identity transpose -> [d, q] drain into oT ----
            def t_unit(h01, qh, qts, tag, eng="v"):
                # h01: True = heads 0/1 pair into oT01, False = head 2
                src = sb01[qh] if h01 else sb2[qh]

                def emit():
                    for i, qt in enumerate(qts):
                        tp = psp.tile([128, 128], BF16,
                                      tag=(tag if i % 2 == 0 else
                                           ("D" if tag == "C" else "C")),
                                      bufs=1, name=f"tp{int(h01)}{qh}{qt}")
                        nc.tensor.transpose(tp, src[:, qt, :], ident)
                        dst_sl = slice(qh * HB + qt * 128,
                                       qh * HB + (qt + 1) * 128)
                        if h01:
                            dst = oT01[:, dst_sl]
                            srcp = tp
                        else:
                            dst = oT2[:, dst_sl]
                            srcp = tp[0:D, :]
                        if eng == "a":
                            nc.scalar.copy(dst, srcp)
                        else:
                            nc.vector.tensor_copy(dst, srcp)
                return emit

            # ---- projection ----
            out_dst = out_d[:].rearrange("(t p) o -> t p o", p=128)
            ostage = {}

            def ost_for(t):
                if t not in ostage:
                    ostage[t] = rwork.tile([128, NI], BF16, tag="ostage",
                                           bufs=6, name=f"ost{t}")
                return ostage[t]

            def proj_piece(t, r, tag, eng):
                # one output region (r=0: cols 0:512, r=1: 512:768) of proj
                # tile t through a single-bank slot
                def emit():
                    n0, n1 = ((0, 512), (512, NI))[r]
                    pp = psp.tile([128, 512], F32, tag=tag, bufs=1,
                                  name=f"pp{t}_{r}")
                    o01 = oT01[:, t * 128:(t + 1) * 128]
                    o2 = oT2[:, t * 128:(t + 1) * 128]
                    nc.tensor.matmul(pp[:, 0:n1 - n0], o01, wp01[:, n0:n1],
                                     start=True, stop=False)
                    nc.tensor.matmul(pp[:, 0:n1 - n0], o2, wp2[:, n0:n1],
                                     start=False, stop=True)
                    ost = ost_for(t)
                    if eng == "a":
                        nc.scalar.copy(ost[:, n0:n1], pp[:, 0:n1 - n0])
                    else:
                        nc.vector.tensor_copy(ost[:, n0:n1], pp[:, 0:n1 - n0])
                    nc.sync.dma_start(out=out_dst[t][:, n0:n1],
                                      in_=ost[:, n0:n1])
                return emit

            def proj_mm1(t, tag):
                pp = psp.tile([128, HB], F32, tag=tag,
                              bufs=(None if tag == "A" else 1),
                              name=f"pp{t}")
                o01 = oT01[:, t * 128:(t + 1) * 128]
                for n0, n1 in ((0, 512), (512, NI)):
                    nc.tensor.matmul(pp[:, n0:n1], o01, wp01[:, n0:n1],
                                     start=True, stop=False)
                return pp

            def proj_mm2(t, pp, eng):
                o2 = oT2[:, t * 128:(t + 1) * 128]
                for n0, n1 in ((0, 512), (512, NI)):
                    nc.tensor.matmul(pp[:, n0:n1], o2, wp2[:, n0:n1],
                                     start=False, stop=True)
                ost = ost_for(t)
                if eng == "a":
                    nc.scalar.copy(ost, pp[:, 0:NI])
                else:
                    nc.vector.tensor_copy(ost, pp[:, 0:NI])
                nc.sync.dma_start(out=out_dst[t], in_=ost)

            def proj_unit(t, tag, eng):
                def emit():
                    proj_mm2(t, proj_mm1(t, tag), eng)
                return emit

            # ---- attention pass ----
            def sc_tile(h, qh, j):
                sc = psp.tile([128, HB], F32, tag="A", name="sc")
                klhs = kk[h][:, j * 128:(j + 1) * 128]
                for n in range(2):
                    sl = slice(qh * HB + n * 512, qh * HB + (n + 1) * 512)
                    nc.tensor.matmul(
                        sc[:, n * 512:(n + 1) * 512], klhs, qq[h][:, sl])
                return sc

            prologues = {}

            def attention_pass(h, qh, filler, last=False, pre_loop=None,
                               nxt=None):
                # acc2 [128 queries, 8 qt, 128]: per query tile, cols 0:64
                # hold out_h[q, d] unnormalized and col 64 the softmax
                # denominator (ones column of vaug); cols 65:128 are pad
                # so each qt slab is 512B and 4 slabs fill a PSUM bank.
                acc2 = psp.tile([128, NQT, 128], F32, tag="B",
                                name=f"acc{h}{qh}", bufs=1)

                sc_q = prologues.pop((h, qh), None)
                if sc_q is None:
                    sc_q = [sc_tile(h, qh, 0)]
                sc_q.append(sc_tile(h, qh, 1))
                if pre_loop is not None:
                    pre_loop()
                fq = list(filler)
                for j in range(NT):
                    # prefire the next pass's first score tile ahead of the
                    # final exp/attV so the next exp(0) has zero gap: its A
                    # slot is free as soon as exp(j=14) has read it
                    if j == NT - 1 and nxt is not None:
                        prologues[nxt] = [sc_tile(*nxt, 0)]
                    ex = expp.tile([128, HB], BF16, tag="exp")
                    nc.scalar.activation(ex, sc_q.pop(0), AF.Exp,
                                         scale=SCALE)
                    if j + 2 < NT:
                        sc_q.append(sc_tile(h, qh, j + 2))
                    # filler between the scores matmul and attV so the PE
                    # has work while exp(j) is still running
                    for _ in range(2):
                        if fq and fq[0][0] <= j:
                            fq.pop(0)[1]()
                        else:
                            break
                    vslab = vaug[:, h, j * 65:j * 65 + 65]
                    for qt in range(NQT):
                        nc.tensor.matmul(
                            acc2[:, qt, 0:65],
                            ex[:, qt * 128:(qt + 1) * 128], vslab,
                            start=(j == 0 and qt % 4 == 0),
                            stop=(j == NT - 1), skip_group_check=True)
                for _, f in fq:  # leftovers (shouldn't happen)
                    f()
                return acc2

            def bcast_last(ap_in, n):
                # view a [..., 1] AP as [..., n] via a stride-0 last dim
                a = [list(p) for p in ap_in.ap]
                assert a[-1][1] == 1
                a[-1] = [0, n]
                return bass.AP(tensor=ap_in.tensor, offset=ap_in.offset,
                               ap=a)

            def norm2(h, qh, acc2):
                # acc2 col 64 holds the per-query denominator: DVE
                # reciprocal, then one fused multiply (denominator
                # broadcast along d with a stride-0 AP) scales all 8 qt
                # into the bf16 [q, d] staging slab and releases the acc
                # banks in a single op.
                rcp = rwork.tile([128, NQT, 1], F32, tag="rcp", bufs=2)
                nc.vector.reciprocal(rcp, acc2[:, :, 64:65])
                dst = sb01[qh] if h < 2 else sb2[qh]
                col = h * D if h < 2 else 0
                nc.vector.tensor_mul(
                    dst[:, :, col:col + D], acc2[:, :, 0:D],
                    bcast_last(rcp[:], D))

            # ---- pass schedule (head-major, h2 last) ----
            # p0 (h0,q0): k(h0) half1 copy + V tiles 8-15
            # p1 (h0,q1): QK^T(h1) pieces
            # p2 (h1,q0): QK^T(h2) pieces 0-1
            # p3 (h1,q1): QK^T(h2) pieces 2-3 + oT01 transpose (q-half 0)
            # p4 (h2,q0): oT01 transpose (q-half 1)
            # p5 (h2,q1): oT2 transpose (q-half 0) + proj of query half 0
            # tail: oT2 transpose (q-half 1) + proj of query half 1
            cd = ("C", "D")

            def pieces(h, entries, j0, dj):
                out = []
                for i, (hf, n) in enumerate(entries):
                    for k, e in enumerate(qkt_piece(h, hf, n, cd[i % 2])):
                        out.append((j0 + dj * i + k, e))
                return out

            # p0: V tiles 4-7 at slots 0/1, then the h0 half-1 QK pieces
            # (keys 1024:1536 must land before sc(8) is emitted at j=6,
            # keys 1536:2048 before sc(12) at j=10 -- emitting them later
            # would deadlock the in-order PE queue), V tiles 8-15 spread
            # so tile t lands before attV consumes it at j=t.
            p0 = ([(0, pv_unit(4, "C")), (1, pv_unit(6, "D"))]
                  + pieces(0, ((1, 0),), 2, 1)
                  + pieces(0, ((1, 1),), 5, 1)
                  + [(8 + i // 2, v_unit(8 + i, cd[(i + 1) % 2]))
                     for i in range(8)])
            p0.sort(key=lambda x: x[0])
            p1 = pieces(1, ((0, 0), (0, 1), (1, 0), (1, 1)), 3, 3)
            p2 = pieces(2, ((0, 0), (0, 1)), 3, 5)
            p3 = pieces(2, ((1, 0), (1, 1)), 3, 5)
            p3 += [(13 + i, t_unit(True, 0, qts, cd[i % 2]))
                   for i, qts in enumerate(((0, 1, 2), (3, 4, 5), (6, 7)))]
            p4 = [(1 + 2 * i, t_unit(True, 1, qts, cd[i % 2]))
                  for i, qts in enumerate(((0, 1, 2), (3, 4, 5), (6, 7)))]
            p5 = [(i, t_unit(False, 0, qts, cd[i % 2]))
                  for i, qts in enumerate(((0, 1, 2), (3, 4, 5), (6, 7)))]
            # all 8 proj tiles of the q0 half, early enough that no filler
            # leaks past the j-loop into the tail's critical DVE window
            p5 += [(2 + (11 * i) // 15, proj_piece(i // 2, i % 2,
                                                   cd[i % 2], "v"))
                   for i in range(16)]
            p5.sort(key=lambda x: x[0])
            acc00 = attention_pass(0, 0, p0, pre_loop=p0_preloop,
                                   nxt=(0, 1))
            acc01 = attention_pass(0, 1, p1,
                                   pre_loop=lambda: norm2(0, 0, acc00),
                                   nxt=(1, 0))
            acc10 = attention_pass(1, 0, p2,
                                   pre_loop=lambda: norm2(0, 1, acc01),
                                   nxt=(1, 1))
            acc11 = attention_pass(1, 1, p3,
                                   pre_loop=lambda: norm2(1, 0, acc10),
                                   nxt=(2, 0))
            acc20 = attention_pass(2, 0, p4,
                                   pre_loop=lambda: norm2(1, 1, acc11),
                                   nxt=(2, 1))
            acc_last = attention_pass(2, 1, p5,
                                      pre_loop=lambda: norm2(2, 0, acc20),
                                      last=True)

            # ---- tail: normalize h2/q1, transpose it into oT2, and
            # project tiles 8-15. Each proj tile 8+qt only needs oT2
            # query tile qt, so the tail runs a per-qt pipeline:
            # scale (DVE) -> PE transpose -> drain (DVE) -> proj matmuls
            # (PE) -> copy split ACT/DVE -> output DMA. ACT is idle after
            # the last exp, so it takes the larger copy half.
            pp_pre = {t: proj_mm1(t, "A") for t in (8, 9)}
            rcp_t = rwork.tile([128, NQT, 1], F32, tag="rcp", bufs=2)
            nc.vector.reciprocal(rcp_t, acc_last[:, :, 64:65])
            nc.vector.tensor_mul(sb2[1][:, :, 0:D], acc_last[:, :, 0:D],
                                 bcast_last(rcp_t[:], D))
            # per-qt pipeline: PE transpose -> drain and a full proj copy
            # on alternating engines -> paired 2-tile output DMAs (each
            # dma_start costs ~630ns of HWDGE, so fewer, bigger wins).
            out_pair = out_d[:].rearrange("(u v p) o -> u p v o",
                                          p=128, v=2)
            for qt in range(NQT):
                dr_a = qt % 2 == 1
                tp = psp.tile([128, 128], BF16, tag=cd[qt % 2], bufs=1,
                              name=f"tpt{qt}")
                nc.tensor.transpose(tp, sb2[1][:, qt, :], ident)
                o2sl = oT2[:, HB + qt * 128:HB + (qt + 1) * 128]
                if dr_a:
                    nc.scalar.copy(o2sl, tp[0:D, :])
                else:
                    nc.vector.tensor_copy(o2sl, tp[0:D, :])
                t8 = 8 + qt
                pp = (pp_pre[t8] if t8 in pp_pre else
                      proj_mm1(t8, ("B", "A", "A", "B", "A", "A")[qt - 2]))
                o2 = oT2[:, t8 * 128:(t8 + 1) * 128]
                for n0, n1 in ((0, 512), (512, NI)):
                    nc.tensor.matmul(pp[:, n0:n1], o2, wp2[:, n0:n1],
                                     start=False, stop=True)
                if qt % 2 == 0:
                    opair = rwork.tile([128, 2, NI], BF16, tag="opair",
                                       bufs=2, name=f"opair{qt // 2}")
                if dr_a:
                    nc.vector.tensor_copy(opair[:, qt % 2, :], pp[:, 0:NI])
                else:
                    nc.scalar.copy(opair[:, qt % 2, :], pp[:, 0:NI])
                if qt % 2 == 1:
                    nc.sync.dma_start(out=out_pair[4 + qt // 2], in_=opair)

    nc.compile()
    return nc


def _get_nc(has_bias: bool):
    if has_bias not in _cache:
        _cache[has_bias] = _build_nc(has_bias)
    return _cache[has_bias]


def kernel(inp, Wqkv, bqkv, Wproj, bproj):
    global last_results
    inp = np.ascontiguousarray(np.asarray(inp, dtype=np.float32))
    Wqkv = np.asarray(Wqkv, dtype=np.float32)
    bqkv = np.asarray(bqkv, dtype=np.float32)
    Wproj = np.asarray(Wproj, dtype=np.float32)
    bproj = np.asarray(bproj, dtype=np.float32)
    assert inp.shape == (NB, S, NI), inp.shape

    has_bias = bool(np.any(bqkv))
    nc = _get_nc(has_bias)

    xTs = [np.ascontiguousarray(inp[b].T).astype(NPBF16) for b in range(NB)]

    in_maps = []
    for core in range(NCORES):
        b = core // CPB
        heads = [(core % CPB) * HPC + i for i in range(HPC)]
        wqk = np.empty((NI, HPC * 128), np.float32)
        wvm = np.zeros((NI, 256), np.float32)
        wp = np.empty((HPC * D, NI), np.float32)
        for i, h in enumerate(heads):
            base = h * 3 * D
            wqk[:, i * 128:i * 128 + D] = Wqkv[:, base:base + D]
            wqk[:, i * 128 + D:(i + 1) * 128] = Wqkv[:, base + D:base + 2 * D]
            wvm[:, i * D:(i + 1) * D] = Wqkv[:, base + 2 * D:base + 3 * D]
            wp[i * D:(i + 1) * D, :] = Wproj[h * D:(h + 1) * D, :]
        m = {"xT": xTs[b], "wqk": wqk.astype(NPBF16),
             "wv": wvm.astype(NPBF16), "wp": wp.astype(NPBF16)}
        if has_bias:
            bqk = np.empty((D, 2 * HPC), np.float32)
            bv = np.empty((HPC * D,), np.float32)
            for i, h in enumerate(heads):
                base = h * 3 * D
                bqk[:, 2 * i] = bqkv[base:base + D]
                bqk[:, 2 * i + 1] = bqkv[base + D:base + 2 * D]
                bv[i * D:(i + 1) * D] = bqkv[base + 2 * D:base + 3 * D]
            m["bqk"] = bqk
            m["bv"] = bv
        in_maps.append(m)

    res = run_bass_kernel_spmd(nc, in_maps, core_ids=list(range(NCORES)))
    last_results = res

    out = np.zeros((NB, S, NI), np.float32)
    for core in range(NCORES):
        out[core // CPB] += np.asarray(res.results[core]["out"],
                                       dtype=np.float32)
    out += bproj
    return out
